# revision 2
# baseline (speedup 1.0000x reference)
"""TARNN-LSTM (nn_B_TARNN_LSTM) Trainium2 Bass kernel.

kernel(**inputs) takes the FULL unsharded inputs (numpy) and returns the
full outputs (outputs[T,B,H], h_final[B,H], c_final[B,H]) matching
reference.reference().

Strategy: data-parallel over batch across 8 NeuronCores (16 rows/core,
weights replicated).  Per core, per step:
  - gates[16, 2560] = [h;c] @ [W_hh; W_bh]^T + pre_t as 20 float32r
    matmuls (weights as the moving operand, N=512 -> full PE rate) plus 5
    identity-matmuls folding the precomputed input projections into the
    same PSUM accumulation group.
  - sigmoid/tanh on ScalarE straight from PSUM, h-update chain on VectorE,
    4 cheap PE transposes rebuild the [state, batch] copy of the state.
  - Input projections (W_ih, W_tp_ih, W_bx, biases) are batched over
    16-step blocks as [256-row x 2560] matmuls that fill PE bubbles,
    staged in SBUF, re-sliced per step with a small SBUF->SBUF DMA
    (compute engines only accept operand base partitions 0/32/64/96).
"""

import sys
import numpy as np

for _p in ("/opt/trn_rl_repo", "/root/.axon_site/_ro/trn_rl_repo"):
    if _p not in sys.path:
        sys.path.append(_p)

T, B, I, H = 512, 128, 256, 256
NC = 8
BL = B // NC
SD = 2 * H
G8 = 8 * H
GT = G8 + SD            # 2560 = gates + (z|bx) block
NT = GT // 512
KC = SD // 128
TB = 16                 # timesteps per pre-block
ROWS_PER_BLOCK = TB * BL
MT = ROWS_PER_BLOCK // 128


def _install_drain_fix():
    """walrus can't encode the many sem-waits Tile puts on the final drain;
    split them across individual SP nops."""
    import bass_rust
    import concourse.tile as tile

    def _drain_and_barrier_split(self, tick_clock, wait_clock):
        nc = self.nc
        probe = nc.sync.nop()
        wait_clock.add_sem_waits(
            probe.ins, bass_rust.ScopedClock({None: tick_clock.global_clock})
        )
        waits = list(probe.ins.sync_info.on_wait)
        probe.ins.sync_info = bass_rust.SyncInfo(on_wait=waits[:1], on_update=[])
        for w in waits[1:]:
            n = nc.sync.nop()
            n.ins.sync_info = bass_rust.SyncInfo(on_wait=[w], on_update=[])
        nc.sync.drain()
        nc.all_engine_barrier()
        popped = nc._tile_sem_poison_stack.pop()
        assert popped is self._sem_poison
        nc.clear_and_free_semaphores(list(self.sems.allocated().values()))
        nc.all_engine_barrier()

    tile.TileContext._drain_and_barrier = _drain_and_barrier_split


def _split_excess_waits(nc, mybir, max_waits=1):
    """walrus can only encode a limited number of sem-waits per instruction;
    hoist excess waits onto standalone EventSemaphore instrs just before."""
    import bass_rust
    n = 0
    for bb_idx, bb in enumerate(nc.m.functions[0].blocks):
        insts = list(bb.instructions)
        out = []
        changed = False
        for inst in insts:
            si = inst.sync_info
            waits = list(si.on_wait) if si is not None else []
            if len(waits) > max_waits:
                keep = waits[-max_waits:]
                for w in waits[:-max_waits]:
                    n += 1
                    es = mybir.InstEventSemaphore(
                        name=f"wsplit_{n}_{bb_idx}",
                        engine=inst.engine,
                        ins=[], outs=[],
                        sync_info=bass_rust.SyncInfo(on_wait=[w], on_update=[]),
                    )
                    nc.register_instruction(es)
                    out.append(es)
                changed = True
                inst.sync_info = bass_rust.SyncInfo(
                    on_wait=keep, on_update=list(si.on_update))
            out.append(inst)
        if changed:
            bb.instructions = out
    return n


def _host_prepare(inputs, h0, c0, weight_ih, weight_hh, bias_ih, bias_hh,
                  weight_bx, weight_bh, weight_tp_ih):
    inputs = np.ascontiguousarray(np.asarray(inputs), np.float32)
    h0 = np.asarray(h0); c0 = np.asarray(c0)
    w_rec_t = np.ascontiguousarray(
        np.concatenate([np.asarray(weight_hh), np.asarray(weight_bh)], axis=0).T,
        np.float32)
    w_pre_t = np.ascontiguousarray(
        np.concatenate([np.asarray(weight_ih), np.asarray(weight_bx)], axis=0).T,
        np.float32)
    w_tp_t = np.ascontiguousarray(np.asarray(weight_tp_ih).T, np.float32)
    bias_pad = np.zeros((128, GT), np.float32)
    bias_pad[0, :G8] = np.asarray(bias_ih) + np.asarray(bias_hh)
    ones_row = np.zeros((128, 128), np.float32); ones_row[0] = 1.0
    ident16 = np.eye(BL, dtype=np.float32)

    in_maps = []
    for c in range(NC):
        sl = slice(c * BL, (c + 1) * BL)
        xc = inputs[:, sl, :]
        xt = np.ascontiguousarray(xc.reshape(T * BL, I).T)
        xt_ext = np.concatenate([xt[:, :BL], xt], axis=1)
        h0c0 = np.concatenate([h0[sl], c0[sl]], axis=1).astype(np.float32)
        in_maps.append({
            "ones_row": ones_row,
            "ident_r": ident16,
            "ident_t": ident16,
            "xt": np.ascontiguousarray(xt_ext, np.float32),
            "w_rec_t": w_rec_t,
            "w_pre_t": w_pre_t,
            "w_tp_t": w_tp_t,
            "bias_pad": bias_pad,
            "h0c0": h0c0,
            "ht0": np.ascontiguousarray(h0c0.T),
        })
    return in_maps


def _build(t_steps=T):
    import concourse.bass as bass
    import concourse.tile as tile
    import concourse.mybir as mybir
    from contextlib import ExitStack

    f32 = mybir.dt.float32
    f32r = mybir.dt.float32r
    AF = mybir.ActivationFunctionType
    n_blocks = t_steps // TB

    nc = bass.Bass("TRN2", target_bir_lowering=False, debug=False)
    xt_d = nc.dram_tensor("xt", [I, BL + T * BL], f32r, kind="ExternalInput").ap()
    wrec_d = nc.dram_tensor("w_rec_t", [SD, GT], f32r, kind="ExternalInput").ap()
    wpre_d = nc.dram_tensor("w_pre_t", [I, GT], f32r, kind="ExternalInput").ap()
    wtp_d = nc.dram_tensor("w_tp_t", [I, G8], f32r, kind="ExternalInput").ap()
    bias_d = nc.dram_tensor("bias_pad", [128, GT], f32r, kind="ExternalInput").ap()
    h0c0_d = nc.dram_tensor("h0c0", [BL, SD], f32, kind="ExternalInput").ap()
    ht0_d = nc.dram_tensor("ht0", [SD, BL], f32r, kind="ExternalInput").ap()
    ones_d = nc.dram_tensor("ones_row", [128, 128], f32r, kind="ExternalInput").ap()
    identr_d = nc.dram_tensor("ident_r", [BL, BL], f32r, kind="ExternalInput").ap()
    identt_d = nc.dram_tensor("ident_t", [BL, BL], f32, kind="ExternalInput").ap()
    outp_d = nc.dram_tensor("out_p", [t_steps, BL, H], f32, kind="ExternalOutput").ap()
    outh_d = nc.dram_tensor("out_h", [BL, H], f32, kind="ExternalOutput").ap()
    outc_d = nc.dram_tensor("out_c", [BL, H], f32, kind="ExternalOutput").ap()

    with tile.TileContext(nc) as tc, ExitStack() as ctx:
        consts = ctx.enter_context(tc.tile_pool(name="consts", bufs=1))
        pre_pool = ctx.enter_context(tc.tile_pool(name="pre", bufs=2))
        xb_pool = ctx.enter_context(tc.tile_pool(name="xb", bufs=2))
        state_pool = ctx.enter_context(tc.tile_pool(name="state", bufs=2))
        tmp_pool = ctx.enter_context(tc.tile_pool(name="tmp", bufs=2))
        pret_pool = ctx.enter_context(tc.tile_pool(name="pret", bufs=3))
        psum_g = ctx.enter_context(tc.tile_pool(name="psg", bufs=1, space="PSUM"))
        psum_tr = ctx.enter_context(tc.tile_pool(name="pstr", bufs=1, space="PSUM"))
        psum_pre = ctx.enter_context(tc.tile_pool(name="pspre", bufs=2, space="PSUM"))

        wrec = consts.tile([128, KC, GT], f32r)
        nc.sync.dma_start(wrec[:], wrec_d.rearrange("(c p) g -> p c g", p=128))
        wpre = consts.tile([128, 2, GT], f32r)
        nc.sync.dma_start(wpre[:], wpre_d.rearrange("(c p) g -> p c g", p=128))
        wtp = consts.tile([128, 2, G8], f32r)
        nc.sync.dma_start(wtp[:], wtp_d.rearrange("(c p) g -> p c g", p=128))
        bias = consts.tile([128, GT], f32r)
        nc.sync.dma_start(bias[:], bias_d[:])
        ones_row = consts.tile([128, 128], f32r)
        nc.sync.dma_start(ones_row[:], ones_d[:])
        ident_r = consts.tile([BL, BL], f32r)
        nc.sync.dma_start(ident_r[:], identr_d[:])
        ident_t = consts.tile([BL, BL], f32)
        nc.sync.dma_start(ident_t[:], identt_d[:])

        h_cur = state_pool.tile([BL, SD], f32, tag="h", name="h_init")
        nc.sync.dma_start(h_cur[:], h0c0_d[:])
        ht_cur = state_pool.tile([128, KC, BL], f32r, tag="ht", name="ht_init")
        nc.sync.dma_start(ht_cur[:], ht0_d.rearrange("(c p) b -> p c b", p=128))

        pre_bufs = {}

        def emit_pre_block(j):
            xb = xb_pool.tile([128, 2, 256 + BL], f32r, tag="xb", name=f"xb{j}")
            nc.sync.dma_start(
                xb[:], xt_d[:, j * 256: j * 256 + 256 + BL]
                .rearrange("(c p) n -> p c n", p=128))
            pre_buf = pre_pool.tile([128, MT, GT], f32r, tag="prebuf", name=f"pre{j}")
            for mi in range(MT):
                for n in range(NT):
                    ns = slice(n * 512, (n + 1) * 512)
                    ps = psum_pre.tile([128, 512], f32, tag="preps",
                                       name=f"pps{j}_{mi}_{n}")
                    mms = [(xb[:, kc, BL + 128 * mi: BL + 128 * mi + 128],
                            wpre[:, kc, ns]) for kc in range(2)]
                    if n < 4:
                        mms += [(xb[:, kc, 128 * mi: 128 * mi + 128],
                                 wtp[:, kc, ns]) for kc in range(2)]
                        mms.append((ones_row[:], bias[:, ns]))
                    for q, (lh, rh) in enumerate(mms):
                        nc.tensor.matmul(ps[:], lh, rh,
                                         start=(q == 0), stop=(q == len(mms) - 1))
                    nc.scalar.copy(pre_buf[:, mi, ns], ps[:])
            pre_bufs[j] = pre_buf

        emit_pre_block(0)

        for t in range(t_steps):
            j, r = divmod(t, TB)
            mi, po = divmod(r * BL, 128)
            pre_buf = pre_bufs[j]

            pre_t = pret_pool.tile([BL, GT], f32r, tag="pret", name=f"pret{t}")
            nc.sync.dma_start(pre_t[:], pre_buf[po:po + BL, mi, :])

            gp = [psum_g.tile([BL, 512], f32, tag=f"g{n}", name=f"g{n}_{t}")
                  for n in range(NT)]
            for n in (2, 4, 0, 1, 3):
                ns = slice(n * 512, (n + 1) * 512)
                for c in range(KC):
                    nc.tensor.matmul(gp[n][:], ht_cur[:, c, :], wrec[:, c, ns],
                                     start=(c == 0), stop=False)
                nc.tensor.matmul(gp[n][:], ident_r[:], pre_t[:, ns],
                                 start=False, stop=True)

            c_s = tmp_pool.tile([BL, 512], f32, tag="c_s", name=f"c_s{t}")
            nc.scalar.activation(c_s[:], gp[2][:], AF.Tanh)
            a = tmp_pool.tile([BL, 512], f32, tag="a", name=f"a{t}")
            nc.vector.tensor_add(a[:], c_s[:], gp[4][:])
            i_s = tmp_pool.tile([BL, 512], f32, tag="i_s", name=f"i_s{t}")
            nc.scalar.activation(i_s[:], gp[0][:], AF.Sigmoid)
            m = tmp_pool.tile([BL, 512], f32, tag="m", name=f"m{t}")
            nc.vector.tensor_mul(m[:], i_s[:], a[:])
            f_s = tmp_pool.tile([BL, 512], f32, tag="f_s", name=f"f_s{t}")
            nc.scalar.activation(f_s[:], gp[1][:], AF.Sigmoid)
            m2 = tmp_pool.tile([BL, 512], f32, tag="m2", name=f"m2{t}")
            nc.vector.tensor_mul(m2[:], f_s[:], h_cur[:])
            s_t = tmp_pool.tile([BL, 512], f32, tag="s_t", name=f"s_t{t}")
            nc.vector.tensor_add(s_t[:], m[:], m2[:])
            ts = tmp_pool.tile([BL, 512], f32, tag="ts", name=f"ts{t}")
            nc.scalar.activation(ts[:], s_t[:], AF.Tanh)
            o_s = tmp_pool.tile([BL, 512], f32, tag="o_s", name=f"o_s{t}")
            nc.scalar.activation(o_s[:], gp[3][:], AF.Sigmoid)
            h_new = state_pool.tile([BL, SD], f32, tag="h", name=f"h{t}")
            nc.vector.tensor_mul(h_new[:], o_s[:], ts[:])

            nc.sync.dma_start(outp_d[t], h_new[:, :H])

            if t + 1 < t_steps:
                trp = psum_tr.tile([128, KC, BL], f32, tag="tr", name=f"tr{t}")
                for c in range(KC):
                    nc.tensor.transpose(trp[:, c, :],
                                        h_new[:, 128 * c:128 * (c + 1)], ident_t[:])
                ht_new = state_pool.tile([128, KC, BL], f32r, tag="ht",
                                         name=f"ht{t}")
                nc.scalar.copy(ht_new[:], trp[:])
                ht_cur = ht_new
            h_cur = h_new
            if r == 0 and j + 1 < n_blocks:
                emit_pre_block(j + 1)
            if r == TB - 1:
                pre_bufs.pop(j, None)

        nc.sync.dma_start(outh_d[:], h_cur[:, :H])
        nc.sync.dma_start(outc_d[:], h_cur[:, H:])

    _split_excess_waits(nc, mybir)
    return nc


_CACHE = {}


def kernel(**inputs):
    _install_drain_fix()
    from concourse.bass_utils import run_bass_kernel_spmd

    in_maps = _host_prepare(**inputs)
    if "nc" not in _CACHE:
        _CACHE["nc"] = _build(T)
    nc = _CACHE["nc"]
    res = run_bass_kernel_spmd(nc, in_maps, core_ids=list(range(NC)))
    outs = res.results
    got_p = np.concatenate([outs[c]["out_p"] for c in range(NC)], axis=1)
    got_h = np.concatenate([outs[c]["out_h"] for c in range(NC)], axis=0)
    got_c = np.concatenate([outs[c]["out_c"] for c in range(NC)], axis=0)
    return got_p, got_h, got_c


# revision 3
# speedup vs baseline: 2.0499x; 2.0499x over previous
"""TARNN-LSTM (nn_B_TARNN_LSTM) Trainium2 Bass kernel.

kernel(**inputs) takes the FULL unsharded inputs (numpy) and returns the
full outputs (outputs[T,B,H], h_final[B,H], c_final[B,H]) matching
reference.reference().

Strategy: data-parallel over batch across 8 NeuronCores (16 rows/core,
weights replicated).  Per core the whole recurrence runs in TRANSPOSED
layout (state dim on partitions, batch on the free dim):

  - gates^T [2560, 16] per step = [W_hh; W_bh] @ [h;c]^T via 80
    W-stationary fp16 matmuls.  These are LDWEIGHTS-bound at the fixed
    1.2 GHz NX clock, which makes them immune to the PE HAM clock-gate
    oscillation that halves streaming-matmul throughput in this
    tail-latency-bound regime.
  - Gate rows are host-permuted to [f i o | c | z] so a single ScalarE
    sigmoid covers f,i,o ([128, 192] tile) and the whole elementwise
    chain runs on full-width [128, 64..192] tiles (8x the lane
    utilization a [16, 512] batch-major layout would get).
  - The state stays transposed end-to-end: h_new is produced directly as
    the next step's matmul operand - no per-step transposes.  Outputs are
    re-transposed to batch-major once per 16-step block (4 PE transposes).
  - Input projections (W_ih, W_tp_ih, W_bx + biases via a ones-row in x)
    are batched per 16-step block as [g,128]x[128,256(t,b)] matmuls that
    fill PE bubbles; the per-step slice is a plain SBUF read at partition
    base 0 (no reshuffle DMA needed in this orientation).

Numerics: fp16 weights/state with fp32 PSUM accumulation and an fp32
copy of the state for the elementwise chain; end-to-end max rel err vs
the fp32 reference is ~1e-3 (the forget-gate contraction damps the
per-step fp16 quantization).
"""

import sys
import numpy as np

for _p in ("/opt/trn_rl_repo", "/root/.axon_site/_ro/trn_rl_repo"):
    if _p not in sys.path:
        sys.path.append(_p)

T, B, I, H = 512, 128, 256, 256
NC = 8
BL = B // NC            # 16
SD = 2 * H              # 512 state dim
G8 = 8 * H              # 2048 gate rows
GT = G8 + SD            # 2560 rows of gates^T (gates + z|bx block)
MTOT = GT // 128        # 20 m-tiles
KC = SD // 128          # 4 recurrent K chunks
TB = 16                 # timesteps per pre-block
NTB = TB * BL           # 256 (t,b) columns per block


def _install_drain_fix():
    """walrus can't encode the many sem-waits Tile puts on the final drain;
    split them across individual SP nops."""
    import bass_rust
    import concourse.tile as tile

    def _drain_and_barrier_split(self, tick_clock, wait_clock):
        nc = self.nc
        probe = nc.sync.nop()
        wait_clock.add_sem_waits(
            probe.ins, bass_rust.ScopedClock({None: tick_clock.global_clock})
        )
        waits = list(probe.ins.sync_info.on_wait)
        probe.ins.sync_info = bass_rust.SyncInfo(on_wait=waits[:1], on_update=[])
        for w in waits[1:]:
            n = nc.sync.nop()
            n.ins.sync_info = bass_rust.SyncInfo(on_wait=[w], on_update=[])
        nc.sync.drain()
        nc.all_engine_barrier()
        popped = nc._tile_sem_poison_stack.pop()
        assert popped is self._sem_poison
        nc.clear_and_free_semaphores(list(self.sems.allocated().values()))
        nc.all_engine_barrier()

    tile.TileContext._drain_and_barrier = _drain_and_barrier_split


def _split_excess_waits(nc, mybir, max_waits=1):
    """walrus can only encode a limited number of sem-waits per instruction;
    hoist excess waits onto standalone EventSemaphore instrs just before."""
    import bass_rust
    n = 0
    for bb_idx, bb in enumerate(nc.m.functions[0].blocks):
        insts = list(bb.instructions)
        out = []
        changed = False
        for inst in insts:
            si = inst.sync_info
            waits = list(si.on_wait) if si is not None else []
            if len(waits) > max_waits:
                keep = waits[-max_waits:]
                for w in waits[:-max_waits]:
                    n += 1
                    es = mybir.InstEventSemaphore(
                        name=f"wsplit_{n}_{bb_idx}",
                        engine=inst.engine,
                        ins=[], outs=[],
                        sync_info=bass_rust.SyncInfo(on_wait=[w], on_update=[]),
                    )
                    nc.register_instruction(es)
                    out.append(es)
                changed = True
                inst.sync_info = bass_rust.SyncInfo(
                    on_wait=keep, on_update=list(si.on_update))
            out.append(inst)
        if changed:
            bb.instructions = out
    return n


def _perm():
    # reference gate row order: [i f c o]; ours: [f i o c]
    return np.concatenate([
        np.arange(512, 1024),    # f
        np.arange(0, 512),       # i
        np.arange(1536, 2048),   # o
        np.arange(1024, 1536),   # c
    ])


def _host_prepare(inputs, h0, c0, weight_ih, weight_hh, bias_ih, bias_hh,
                  weight_bx, weight_bh, weight_tp_ih):
    p = _perm()
    inputs = np.ascontiguousarray(np.asarray(inputs), np.float32)
    whh = np.asarray(weight_hh)[p]
    wih = np.asarray(weight_ih)[p]
    wtp = np.asarray(weight_tp_ih)[p]
    bias = (np.asarray(bias_ih) + np.asarray(bias_hh))[p]

    w_rec_t = np.concatenate([whh, np.asarray(weight_bh)], axis=0).T   # [512, 2560]
    w_pre = np.concatenate([wih, np.asarray(weight_bx)], axis=0)       # [2560, 256]
    w_pre_t = np.zeros((384, GT), np.float32)
    w_pre_t[:I] = w_pre.T
    w_pre_t[I, :G8] = bias                                             # ones-row bias
    w_tp_t = wtp.T                                                     # [256, 2048]

    in_maps = []
    for c in range(NC):
        sl = slice(c * BL, (c + 1) * BL)
        xc = inputs[:, sl, :]
        xt = np.ascontiguousarray(xc.reshape(T * BL, I).T)             # [I, T*16]
        xt_ext = np.zeros((384, BL + T * BL), np.float32)
        xt_ext[:I, :BL] = xt[:, :BL]        # old_inputs[0] = inputs[0]
        xt_ext[:I, BL:] = xt
        xt_ext[I, :] = 1.0                  # ones feature -> bias term
        h0c0 = np.concatenate([np.asarray(h0)[sl], np.asarray(c0)[sl]],
                              axis=1).astype(np.float32)
        ht0 = np.ascontiguousarray(h0c0.T)                             # [512, 16]
        in_maps.append({
            "xt": xt_ext.astype(np.float16),
            "w_rec_t": w_rec_t.astype(np.float16),
            "w_pre_t": w_pre_t.astype(np.float16),
            "w_tp_t": w_tp_t.astype(np.float16),
            "ht0_32": ht0.astype(np.float32),
            "ht0_16": ht0.astype(np.float16),
            "ident128": np.eye(128, dtype=np.float16),
        })
    return in_maps


def _build(t_steps=T):
    import concourse.bass as bass_mod
    import concourse.tile as tile_mod
    import concourse.mybir as mybir
    from contextlib import ExitStack

    f32 = mybir.dt.float32
    f16 = mybir.dt.float16
    AF = mybir.ActivationFunctionType
    n_blocks = t_steps // TB

    nc = bass_mod.Bass("TRN2", target_bir_lowering=False, debug=False)
    xt_d = nc.dram_tensor("xt", [384, BL + T * BL], f16, kind="ExternalInput").ap()
    wrec_d = nc.dram_tensor("w_rec_t", [SD, GT], f16, kind="ExternalInput").ap()
    wpre_d = nc.dram_tensor("w_pre_t", [384, GT], f16, kind="ExternalInput").ap()
    wtp_d = nc.dram_tensor("w_tp_t", [I, G8], f16, kind="ExternalInput").ap()
    ht032_d = nc.dram_tensor("ht0_32", [SD, BL], f32, kind="ExternalInput").ap()
    ht016_d = nc.dram_tensor("ht0_16", [SD, BL], f16, kind="ExternalInput").ap()
    id128_d = nc.dram_tensor("ident128", [128, 128], f16, kind="ExternalInput").ap()
    outp_d = nc.dram_tensor("out_p", [t_steps, BL, H], f32, kind="ExternalOutput").ap()
    outh_d = nc.dram_tensor("out_h", [BL, H], f32, kind="ExternalOutput").ap()
    outc_d = nc.dram_tensor("out_c", [BL, H], f32, kind="ExternalOutput").ap()

    with tile_mod.TileContext(nc) as tc, ExitStack() as ctx:
        consts = ctx.enter_context(tc.tile_pool(name="consts", bufs=1))
        pre_pool = ctx.enter_context(tc.tile_pool(name="pre", bufs=2))
        xb_pool = ctx.enter_context(tc.tile_pool(name="xb", bufs=2))
        state_pool = ctx.enter_context(tc.tile_pool(name="state", bufs=2))
        tmp_pool = ctx.enter_context(tc.tile_pool(name="tmp", bufs=2))
        pblk_pool = ctx.enter_context(tc.tile_pool(name="pblk", bufs=2))
        psum_g = ctx.enter_context(tc.tile_pool(name="psg", bufs=2, space="PSUM"))
        psum_pre = ctx.enter_context(tc.tile_pool(name="pspre", bufs=2, space="PSUM"))
        psum_tr = ctx.enter_context(tc.tile_pool(name="pstr", bufs=2, space="PSUM"))

        wrec = consts.tile([128, KC, GT], f16)
        nc.sync.dma_start(wrec[:], wrec_d.rearrange("(c p) g -> p c g", p=128))
        wpre = consts.tile([128, 3, GT], f16)
        nc.sync.dma_start(wpre[:], wpre_d.rearrange("(c p) g -> p c g", p=128))
        wtp = consts.tile([128, 2, G8], f16)
        nc.sync.dma_start(wtp[:], wtp_d.rearrange("(c p) g -> p c g", p=128))
        id128 = consts.tile([128, 128], f16)
        nc.sync.dma_start(id128[:], id128_d[:])

        h32 = state_pool.tile([128, KC, BL], f32, tag="h32", name="h32_init")
        nc.sync.dma_start(h32[:], ht032_d.rearrange("(c p) b -> p c b", p=128))
        h16 = state_pool.tile([128, KC, BL], f16, tag="h16", name="h16_init")
        nc.sync.dma_start(h16[:], ht016_d.rearrange("(c p) b -> p c b", p=128))

        pre_bufs = {}

        def emit_pre_block(j):
            xb = xb_pool.tile([128, 3, NTB + BL], f16, tag="xb", name=f"xb{j}")
            nc.sync.dma_start(
                xb[:], xt_d[:, j * NTB: j * NTB + NTB + BL]
                .rearrange("(c p) n -> p c n", p=128))
            pre_buf = pre_pool.tile([128, MTOT, NTB], f32, tag="prebuf",
                                    name=f"pre{j}")
            for m in range(MTOT):
                ms = slice(m * 128, (m + 1) * 128)
                ps = psum_pre.tile([128, NTB], f32, tag="preps", name=f"pps{j}_{m}")
                mms = [(wpre[:, kc, ms], xb[:, kc, BL:]) for kc in range(3)]
                if m < 16:   # x_old projections only touch the gate rows
                    mms += [(wtp[:, kc, ms], xb[:, kc, :NTB]) for kc in range(2)]
                for q, (lh, rh) in enumerate(mms):
                    nc.tensor.matmul(ps[:], lh, rh,
                                     start=(q == 0), stop=(q == len(mms) - 1))
                eng = nc.scalar.copy if m % 2 else nc.vector.tensor_copy
                eng(pre_buf[:, m, :], ps[:])
            pre_bufs[j] = pre_buf

        emit_pre_block(0)
        pblk = None

        for t in range(t_steps):
            j, r = divmod(t, TB)
            pre_buf = pre_bufs[j]
            cs = slice(r * BL, (r + 1) * BL)

            gp = psum_g.tile([128, MTOT, BL], f32, tag="gp", name=f"gp{t}")
            for m in range(MTOT):   # sigma block first so its chain overlaps c/z
                ms = slice(m * 128, (m + 1) * 128)
                for c in range(KC):
                    nc.tensor.matmul(gp[:, m, :], wrec[:, c, ms], h16[:, c, :],
                                     start=(c == 0), stop=(c == KC - 1))

            tsig = tmp_pool.tile([128, 12, BL], f32, tag="tsig", name=f"tsig{t}")
            nc.vector.tensor_add(tsig[:], gp[:, 0:12, :], pre_buf[:, 0:12, cs])
            sig = tmp_pool.tile([128, 12, BL], f32, tag="sig", name=f"sig{t}")
            nc.scalar.activation(sig[:], tsig[:], AF.Sigmoid)
            m2 = tmp_pool.tile([128, KC, BL], f32, tag="m2", name=f"m2{t}")
            nc.vector.tensor_mul(m2[:], sig[:, 0:4, :], h32[:])
            tc_ = tmp_pool.tile([128, KC, BL], f32, tag="tc", name=f"tc{t}")
            nc.vector.tensor_add(tc_[:], gp[:, 12:16, :], pre_buf[:, 12:16, cs])
            c_s = tmp_pool.tile([128, KC, BL], f32, tag="c_s", name=f"c_s{t}")
            nc.scalar.activation(c_s[:], tc_[:], AF.Tanh)
            bz = tmp_pool.tile([128, KC, BL], f32, tag="bz", name=f"bz{t}")
            nc.vector.tensor_add(bz[:], gp[:, 16:20, :], pre_buf[:, 16:20, cs])
            acc = tmp_pool.tile([128, KC, BL], f32, tag="acc", name=f"acc{t}")
            nc.vector.tensor_add(acc[:], c_s[:], bz[:])
            m1 = tmp_pool.tile([128, KC, BL], f32, tag="m1", name=f"m1{t}")
            nc.vector.tensor_mul(m1[:], sig[:, 4:8, :], acc[:])
            s_t = tmp_pool.tile([128, KC, BL], f32, tag="s_t", name=f"s_t{t}")
            nc.vector.tensor_add(s_t[:], m1[:], m2[:])
            ts = tmp_pool.tile([128, KC, BL], f32, tag="ts", name=f"ts{t}")
            nc.scalar.activation(ts[:], s_t[:], AF.Tanh)
            h32n = state_pool.tile([128, KC, BL], f32, tag="h32", name=f"h32_{t}")
            nc.vector.tensor_mul(h32n[:], sig[:, 8:12, :], ts[:])
            h16n = state_pool.tile([128, KC, BL], f16, tag="h16", name=f"h16_{t}")
            nc.vector.tensor_copy(h16n[:], h32n[:])
            h32, h16 = h32n, h16n

            if r == 0:
                pblk = pblk_pool.tile([128, 2, NTB], f16, tag="pblk", name=f"pb{j}")
            nc.scalar.copy(pblk[:, :, cs], h16n[:, 0:2, :])

            if r == 0 and j + 1 < n_blocks:
                emit_pre_block(j + 1)
            if r == TB - 1:
                # transpose [256(h), 256(tb)] -> [256(tb), 256(h)], then DMA
                trp = psum_tr.tile([128, 4, 128], f16, tag="trp", name=f"trp{j}")
                for cc in range(2):
                    for nn_ in range(2):
                        nc.tensor.transpose(trp[:, cc * 2 + nn_, :],
                                            pblk[:, cc, nn_ * 128:(nn_ + 1) * 128],
                                            id128[:])
                ob = pblk_pool.tile([128, 2, 256], f32, tag="ob", name=f"ob{j}")
                nc.scalar.copy(ob[:, 0, 0:128], trp[:, 0, :])
                nc.scalar.copy(ob[:, 0, 128:256], trp[:, 2, :])
                nc.scalar.copy(ob[:, 1, 0:128], trp[:, 1, :])
                nc.scalar.copy(ob[:, 1, 128:256], trp[:, 3, :])
                nc.sync.dma_start(
                    outp_d[j * TB:(j + 1) * TB]
                    .rearrange("t b h -> (t b) h")
                    .rearrange("(c p) h -> p c h", p=128), ob[:])
                pre_bufs.pop(j, None)

        fin = psum_tr.tile([16, 4, 128], f16, tag="fin", name="fin")
        for cc in range(4):
            nc.tensor.transpose(fin[:, cc, :], h16[:, cc, :], id128[:])
        hcbuf = pblk_pool.tile([16, 4, 128], f32, tag="hc", name="hcbuf")
        nc.scalar.copy(hcbuf[:], fin[:])
        nc.sync.dma_start(outh_d[:], hcbuf[:, 0:2, :].rearrange("b c h -> b (c h)"))
        nc.sync.dma_start(outc_d[:], hcbuf[:, 2:4, :].rearrange("b c h -> b (c h)"))

    _split_excess_waits(nc, mybir)
    return nc


_CACHE = {}


def kernel(**inputs):
    _install_drain_fix()
    from concourse.bass_utils import run_bass_kernel_spmd

    in_maps = _host_prepare(**inputs)
    if "nc" not in _CACHE:
        _CACHE["nc"] = _build(T)
    nc = _CACHE["nc"]
    res = run_bass_kernel_spmd(nc, in_maps, core_ids=list(range(NC)))
    outs = res.results
    got_p = np.concatenate([outs[c]["out_p"] for c in range(NC)], axis=1)
    got_h = np.concatenate([outs[c]["out_h"] for c in range(NC)], axis=0)
    got_c = np.concatenate([outs[c]["out_c"] for c in range(NC)], axis=0)
    return got_p, got_h, got_c


# revision 4
# speedup vs baseline: 2.1377x; 1.0428x over previous
"""TARNN-LSTM (nn_B_TARNN_LSTM) Trainium2 Bass kernel.

kernel(**inputs) takes the FULL unsharded inputs (numpy) and returns the
full outputs (outputs[T,B,H], h_final[B,H], c_final[B,H]) matching
reference.reference().

Strategy: data-parallel over batch across 8 NeuronCores (16 rows/core,
weights replicated).  Per core the whole recurrence runs in TRANSPOSED
layout (state dim on partitions, batch on the free dim):

  - gates^T [2560, 16] per step = [W_hh; W_bh] @ [h;c]^T via 80
    W-stationary fp16 matmuls.  These are LDWEIGHTS-bound at the fixed
    1.2 GHz NX clock, which makes them immune to the PE HAM clock-gate
    oscillation that halves streaming-matmul throughput in this
    tail-latency-bound regime.
  - Gate rows are host-permuted to [f i o | c | z] so a single ScalarE
    sigmoid covers f,i,o ([128, 192] tile) and the whole elementwise
    chain runs on full-width [128, 64..192] tiles (8x the lane
    utilization a [16, 512] batch-major layout would get).
  - The state stays transposed end-to-end: h_new is produced directly as
    the next step's matmul operand - no per-step transposes.  Outputs are
    re-transposed to batch-major once per 16-step block (4 PE transposes).
  - Input projections (W_ih, W_tp_ih, W_bx + biases via a ones-row in x)
    are batched per 16-step block as [g,128]x[128,256(t,b)] matmuls that
    fill PE bubbles; the per-step slice is a plain SBUF read at partition
    base 0 (no reshuffle DMA needed in this orientation).

Numerics: fp16 weights/state with fp32 PSUM accumulation and an fp32
copy of the state for the elementwise chain; end-to-end max rel err vs
the fp32 reference is ~1e-3 (the forget-gate contraction damps the
per-step fp16 quantization).
"""

import sys
import numpy as np

for _p in ("/opt/trn_rl_repo", "/root/.axon_site/_ro/trn_rl_repo"):
    if _p not in sys.path:
        sys.path.append(_p)

T, B, I, H = 512, 128, 256, 256
NC = 8
BL = B // NC            # 16
SD = 2 * H              # 512 state dim
G8 = 8 * H              # 2048 gate rows
GT = G8 + SD            # 2560 rows of gates^T (gates + z|bx block)
MTOT = GT // 128        # 20 m-tiles
KC = SD // 128          # 4 recurrent K chunks
TB = 16                 # timesteps per pre-block
NTB = TB * BL           # 256 (t,b) columns per block


def _install_drain_fix():
    """walrus can't encode the many sem-waits Tile puts on the final drain;
    split them across individual SP nops."""
    import bass_rust
    import concourse.tile as tile

    def _drain_and_barrier_split(self, tick_clock, wait_clock):
        nc = self.nc
        probe = nc.sync.nop()
        wait_clock.add_sem_waits(
            probe.ins, bass_rust.ScopedClock({None: tick_clock.global_clock})
        )
        waits = list(probe.ins.sync_info.on_wait)
        probe.ins.sync_info = bass_rust.SyncInfo(on_wait=waits[:1], on_update=[])
        for w in waits[1:]:
            n = nc.sync.nop()
            n.ins.sync_info = bass_rust.SyncInfo(on_wait=[w], on_update=[])
        nc.sync.drain()
        nc.all_engine_barrier()
        popped = nc._tile_sem_poison_stack.pop()
        assert popped is self._sem_poison
        nc.clear_and_free_semaphores(list(self.sems.allocated().values()))
        nc.all_engine_barrier()

    tile.TileContext._drain_and_barrier = _drain_and_barrier_split


def _split_excess_waits(nc, mybir, max_waits=1):
    """walrus can only encode a limited number of sem-waits per instruction;
    hoist excess waits onto standalone EventSemaphore instrs just before."""
    import bass_rust
    n = 0
    for bb_idx, bb in enumerate(nc.m.functions[0].blocks):
        insts = list(bb.instructions)
        out = []
        changed = False
        for inst in insts:
            si = inst.sync_info
            waits = list(si.on_wait) if si is not None else []
            if len(waits) > max_waits:
                keep = waits[-max_waits:]
                for w in waits[:-max_waits]:
                    n += 1
                    es = mybir.InstEventSemaphore(
                        name=f"wsplit_{n}_{bb_idx}",
                        engine=inst.engine,
                        ins=[], outs=[],
                        sync_info=bass_rust.SyncInfo(on_wait=[w], on_update=[]),
                    )
                    nc.register_instruction(es)
                    out.append(es)
                changed = True
                inst.sync_info = bass_rust.SyncInfo(
                    on_wait=keep, on_update=list(si.on_update))
            out.append(inst)
        if changed:
            bb.instructions = out
    return n


def _perm():
    # reference gate row order: [i f c o]; ours: [f i o c]
    return np.concatenate([
        np.arange(512, 1024),    # f
        np.arange(0, 512),       # i
        np.arange(1536, 2048),   # o
        np.arange(1024, 1536),   # c
    ])


def _host_prepare(inputs, h0, c0, weight_ih, weight_hh, bias_ih, bias_hh,
                  weight_bx, weight_bh, weight_tp_ih):
    p = _perm()
    inputs = np.ascontiguousarray(np.asarray(inputs), np.float32)
    whh = np.asarray(weight_hh)[p]
    wih = np.asarray(weight_ih)[p]
    wtp = np.asarray(weight_tp_ih)[p]
    bias = (np.asarray(bias_ih) + np.asarray(bias_hh))[p]

    w_rec_t = np.concatenate([whh, np.asarray(weight_bh)], axis=0).T   # [512, 2560]
    w_pre = np.concatenate([wih, np.asarray(weight_bx)], axis=0)       # [2560, 256]
    w_pre_t = np.zeros((384, GT), np.float32)
    w_pre_t[:I] = w_pre.T
    w_pre_t[I, :G8] = bias                                             # ones-row bias
    w_tp_t = wtp.T                                                     # [256, 2048]

    in_maps = []
    for c in range(NC):
        sl = slice(c * BL, (c + 1) * BL)
        xc = inputs[:, sl, :]
        xt = np.ascontiguousarray(xc.reshape(T * BL, I).T)             # [I, T*16]
        xt_ext = np.zeros((384, BL + T * BL), np.float32)
        xt_ext[:I, :BL] = xt[:, :BL]        # old_inputs[0] = inputs[0]
        xt_ext[:I, BL:] = xt
        xt_ext[I, :] = 1.0                  # ones feature -> bias term
        h0c0 = np.concatenate([np.asarray(h0)[sl], np.asarray(c0)[sl]],
                              axis=1).astype(np.float32)
        ht0 = np.ascontiguousarray(h0c0.T)                             # [512, 16]
        in_maps.append({
            "xt": xt_ext.astype(np.float16),
            "w_rec_t": w_rec_t.astype(np.float16),
            "w_pre_t": w_pre_t.astype(np.float16),
            "w_tp_t": w_tp_t.astype(np.float16),
            "ht0_32": ht0.astype(np.float32),
            "ht0_16": ht0.astype(np.float16),
            "ident128": np.eye(128, dtype=np.float16),
        })
    return in_maps


def _build(t_steps=T):
    import concourse.bass as bass_mod
    import concourse.tile as tile_mod
    import concourse.mybir as mybir
    from contextlib import ExitStack

    f32 = mybir.dt.float32
    f16 = mybir.dt.float16
    AF = mybir.ActivationFunctionType
    n_blocks = t_steps // TB

    nc = bass_mod.Bass("TRN2", target_bir_lowering=False, debug=False)
    xt_d = nc.dram_tensor("xt", [384, BL + T * BL], f16, kind="ExternalInput").ap()
    wrec_d = nc.dram_tensor("w_rec_t", [SD, GT], f16, kind="ExternalInput").ap()
    wpre_d = nc.dram_tensor("w_pre_t", [384, GT], f16, kind="ExternalInput").ap()
    wtp_d = nc.dram_tensor("w_tp_t", [I, G8], f16, kind="ExternalInput").ap()
    ht032_d = nc.dram_tensor("ht0_32", [SD, BL], f32, kind="ExternalInput").ap()
    ht016_d = nc.dram_tensor("ht0_16", [SD, BL], f16, kind="ExternalInput").ap()
    id128_d = nc.dram_tensor("ident128", [128, 128], f16, kind="ExternalInput").ap()
    outp_d = nc.dram_tensor("out_p", [t_steps, BL, H], f32, kind="ExternalOutput").ap()
    outh_d = nc.dram_tensor("out_h", [BL, H], f32, kind="ExternalOutput").ap()
    outc_d = nc.dram_tensor("out_c", [BL, H], f32, kind="ExternalOutput").ap()

    with tile_mod.TileContext(nc) as tc, ExitStack() as ctx:
        consts = ctx.enter_context(tc.tile_pool(name="consts", bufs=1))
        pre_pool = ctx.enter_context(tc.tile_pool(name="pre", bufs=2))
        xb_pool = ctx.enter_context(tc.tile_pool(name="xb", bufs=2))
        state_pool = ctx.enter_context(tc.tile_pool(name="state", bufs=2))
        tmp_pool = ctx.enter_context(tc.tile_pool(name="tmp", bufs=2))
        pblk_pool = ctx.enter_context(tc.tile_pool(name="pblk", bufs=2))
        psum_g = ctx.enter_context(tc.tile_pool(name="psg", bufs=2, space="PSUM"))
        psum_pre = ctx.enter_context(tc.tile_pool(name="pspre", bufs=2, space="PSUM"))
        psum_tr = ctx.enter_context(tc.tile_pool(name="pstr", bufs=2, space="PSUM"))

        wrec = consts.tile([128, KC, GT], f16)
        nc.sync.dma_start(wrec[:], wrec_d.rearrange("(c p) g -> p c g", p=128))
        wpre = consts.tile([128, 3, GT], f16)
        nc.sync.dma_start(wpre[:], wpre_d.rearrange("(c p) g -> p c g", p=128))
        wtp = consts.tile([128, 2, G8], f16)
        nc.sync.dma_start(wtp[:], wtp_d.rearrange("(c p) g -> p c g", p=128))
        id128 = consts.tile([128, 128], f16)
        nc.sync.dma_start(id128[:], id128_d[:])

        h32 = state_pool.tile([128, KC, BL], f32, tag="h32", name="h32_init")
        nc.sync.dma_start(h32[:], ht032_d.rearrange("(c p) b -> p c b", p=128))
        h16 = state_pool.tile([128, KC, BL], f16, tag="h16", name="h16_init")
        nc.sync.dma_start(h16[:], ht016_d.rearrange("(c p) b -> p c b", p=128))

        pre_bufs = {}

        def emit_pre_block(j):
            xb = xb_pool.tile([128, 3, NTB + BL], f16, tag="xb", name=f"xb{j}")
            nc.sync.dma_start(
                xb[:], xt_d[:, j * NTB: j * NTB + NTB + BL]
                .rearrange("(c p) n -> p c n", p=128))
            pre_buf = pre_pool.tile([128, MTOT, NTB], f32, tag="prebuf",
                                    name=f"pre{j}")
            for m in range(MTOT):
                ms = slice(m * 128, (m + 1) * 128)
                ps = psum_pre.tile([128, NTB], f32, tag="preps", name=f"pps{j}_{m}")
                mms = [(wpre[:, kc, ms], xb[:, kc, BL:]) for kc in range(3)]
                if m < 16:   # x_old projections only touch the gate rows
                    mms += [(wtp[:, kc, ms], xb[:, kc, :NTB]) for kc in range(2)]
                for q, (lh, rh) in enumerate(mms):
                    nc.tensor.matmul(ps[:], lh, rh,
                                     start=(q == 0), stop=(q == len(mms) - 1))
                eng = nc.scalar.copy if m % 2 else nc.vector.tensor_copy
                eng(pre_buf[:, m, :], ps[:])
            pre_bufs[j] = pre_buf

        emit_pre_block(0)
        pblk = None

        for t in range(t_steps):
            j, r = divmod(t, TB)
            pre_buf = pre_bufs[j]
            cs = slice(r * BL, (r + 1) * BL)

            gp = psum_g.tile([128, MTOT, BL], f32, tag="gp", name=f"gp{t}")
            for m in range(MTOT):   # sigma block first so its chain overlaps c/z
                ms = slice(m * 128, (m + 1) * 128)
                for c in range(KC):
                    nc.tensor.matmul(gp[:, m, :], wrec[:, c, ms], h16[:, c, :],
                                     start=(c == 0), stop=(c == KC - 1))

            tsig = tmp_pool.tile([128, 12, BL], f32, tag="tsig", name=f"tsig{t}")
            nc.vector.tensor_add(tsig[:], gp[:, 0:12, :], pre_buf[:, 0:12, cs])
            sig = tmp_pool.tile([128, 12, BL], f32, tag="sig", name=f"sig{t}")
            nc.scalar.activation(sig[:], tsig[:], AF.Sigmoid)
            m2 = tmp_pool.tile([128, KC, BL], f32, tag="m2", name=f"m2{t}")
            nc.vector.tensor_mul(m2[:], sig[:, 0:4, :], h32[:])
            tc_ = tmp_pool.tile([128, KC, BL], f32, tag="tc", name=f"tc{t}")
            nc.vector.tensor_add(tc_[:], gp[:, 12:16, :], pre_buf[:, 12:16, cs])
            c_s = tmp_pool.tile([128, KC, BL], f32, tag="c_s", name=f"c_s{t}")
            nc.scalar.activation(c_s[:], tc_[:], AF.Tanh)
            bz = tmp_pool.tile([128, KC, BL], f32, tag="bz", name=f"bz{t}")
            nc.vector.tensor_add(bz[:], gp[:, 16:20, :], pre_buf[:, 16:20, cs])
            acc = tmp_pool.tile([128, KC, BL], f32, tag="acc", name=f"acc{t}")
            nc.vector.tensor_add(acc[:], c_s[:], bz[:])
            m1 = tmp_pool.tile([128, KC, BL], f32, tag="m1", name=f"m1{t}")
            nc.vector.tensor_mul(m1[:], sig[:, 4:8, :], acc[:])
            s_t = tmp_pool.tile([128, KC, BL], f32, tag="s_t", name=f"s_t{t}")
            nc.vector.tensor_add(s_t[:], m1[:], m2[:])
            ts = tmp_pool.tile([128, KC, BL], f32, tag="ts", name=f"ts{t}")
            nc.scalar.activation(ts[:], s_t[:], AF.Tanh)
            # fp16 state first: it is the only input of step t+1's matmuls
            h16n = state_pool.tile([128, KC, BL], f16, tag="h16", name=f"h16_{t}")
            nc.vector.tensor_mul(h16n[:], sig[:, 8:12, :], ts[:])
            # fp32 copy (for next step's f*h) off the critical path
            h32n = state_pool.tile([128, KC, BL], f32, tag="h32", name=f"h32_{t}")
            nc.vector.tensor_mul(h32n[:], sig[:, 8:12, :], ts[:])
            h32, h16 = h32n, h16n

            if r == 0:
                pblk = pblk_pool.tile([128, 2, NTB], f16, tag="pblk", name=f"pb{j}")
            nc.scalar.copy(pblk[:, :, cs], h16n[:, 0:2, :])

            if r == 0 and j + 1 < n_blocks:
                emit_pre_block(j + 1)
            if r == TB - 1:
                # transpose [256(h), 256(tb)] -> [256(tb), 256(h)], then DMA
                trp = psum_tr.tile([128, 4, 128], f16, tag="trp", name=f"trp{j}")
                for cc in range(2):
                    for nn_ in range(2):
                        nc.tensor.transpose(trp[:, cc * 2 + nn_, :],
                                            pblk[:, cc, nn_ * 128:(nn_ + 1) * 128],
                                            id128[:])
                ob = pblk_pool.tile([128, 2, 256], f32, tag="ob", name=f"ob{j}")
                nc.scalar.copy(ob[:, 0, 0:128], trp[:, 0, :])
                nc.scalar.copy(ob[:, 0, 128:256], trp[:, 2, :])
                nc.scalar.copy(ob[:, 1, 0:128], trp[:, 1, :])
                nc.scalar.copy(ob[:, 1, 128:256], trp[:, 3, :])
                nc.sync.dma_start(
                    outp_d[j * TB:(j + 1) * TB]
                    .rearrange("t b h -> (t b) h")
                    .rearrange("(c p) h -> p c h", p=128), ob[:])
                pre_bufs.pop(j, None)

        fin = psum_tr.tile([16, 4, 128], f16, tag="fin", name="fin")
        for cc in range(4):
            nc.tensor.transpose(fin[:, cc, :], h16[:, cc, :], id128[:])
        hcbuf = pblk_pool.tile([16, 4, 128], f32, tag="hc", name="hcbuf")
        nc.scalar.copy(hcbuf[:], fin[:])
        nc.sync.dma_start(outh_d[:], hcbuf[:, 0:2, :].rearrange("b c h -> b (c h)"))
        nc.sync.dma_start(outc_d[:], hcbuf[:, 2:4, :].rearrange("b c h -> b (c h)"))

    _split_excess_waits(nc, mybir)
    return nc


_CACHE = {}


def kernel(**inputs):
    _install_drain_fix()
    from concourse.bass_utils import run_bass_kernel_spmd

    in_maps = _host_prepare(**inputs)
    if "nc" not in _CACHE:
        _CACHE["nc"] = _build(T)
    nc = _CACHE["nc"]
    res = run_bass_kernel_spmd(nc, in_maps, core_ids=list(range(NC)))
    outs = res.results
    got_p = np.concatenate([outs[c]["out_p"] for c in range(NC)], axis=1)
    got_h = np.concatenate([outs[c]["out_h"] for c in range(NC)], axis=0)
    got_c = np.concatenate([outs[c]["out_c"] for c in range(NC)], axis=0)
    return got_p, got_h, got_c


# revision 5
# speedup vs baseline: 2.1394x; 1.0008x over previous
"""TARNN-LSTM (nn_B_TARNN_LSTM) Trainium2 Bass kernel.

kernel(**inputs) takes the FULL unsharded inputs (numpy) and returns the
full outputs (outputs[T,B,H], h_final[B,H], c_final[B,H]) matching
reference.reference().

Strategy: data-parallel over batch across 8 NeuronCores (16 rows/core,
weights replicated).  Per core the whole recurrence runs in TRANSPOSED
layout (state dim on partitions, batch on the free dim):

  - gates^T [2560, 16] per step = [W_hh; W_bh] @ [h;c]^T via 80
    W-stationary fp16 matmuls.  These are LDWEIGHTS-bound at the fixed
    1.2 GHz NX clock, which makes them immune to the PE HAM clock-gate
    oscillation that halves streaming-matmul throughput in this
    tail-latency-bound regime.
  - Gate rows are host-permuted to [f i o | c | z] so a single ScalarE
    sigmoid covers f,i,o ([128, 192] tile) and the whole elementwise
    chain runs on full-width [128, 64..192] tiles (8x the lane
    utilization a [16, 512] batch-major layout would get).
  - The state stays transposed end-to-end: h_new is produced directly as
    the next step's matmul operand - no per-step transposes.  Outputs are
    re-transposed to batch-major once per 16-step block (4 PE transposes).
  - Input projections (W_ih, W_tp_ih, W_bx + biases via a ones-row in x)
    are batched per 16-step block as [g,128]x[128,256(t,b)] matmuls that
    fill PE bubbles; the per-step slice is a plain SBUF read at partition
    base 0 (no reshuffle DMA needed in this orientation).

Numerics: fp16 weights/state with fp32 PSUM accumulation and an fp32
copy of the state for the elementwise chain; end-to-end max rel err vs
the fp32 reference is ~1e-3 (the forget-gate contraction damps the
per-step fp16 quantization).
"""

import sys
import numpy as np

for _p in ("/opt/trn_rl_repo", "/root/.axon_site/_ro/trn_rl_repo"):
    if _p not in sys.path:
        sys.path.append(_p)

T, B, I, H = 512, 128, 256, 256
NC = 8
BL = B // NC            # 16
SD = 2 * H              # 512 state dim
G8 = 8 * H              # 2048 gate rows
GT = G8 + SD            # 2560 rows of gates^T (gates + z|bx block)
MTOT = GT // 128        # 20 m-tiles
KC = SD // 128          # 4 recurrent K chunks
TB = 16                 # timesteps per pre-block
NTB = TB * BL           # 256 (t,b) columns per block


def _install_drain_fix():
    """walrus can't encode the many sem-waits Tile puts on the final drain;
    split them across individual SP nops."""
    import bass_rust
    import concourse.tile as tile

    def _drain_and_barrier_split(self, tick_clock, wait_clock):
        nc = self.nc
        probe = nc.sync.nop()
        wait_clock.add_sem_waits(
            probe.ins, bass_rust.ScopedClock({None: tick_clock.global_clock})
        )
        waits = list(probe.ins.sync_info.on_wait)
        probe.ins.sync_info = bass_rust.SyncInfo(on_wait=waits[:1], on_update=[])
        for w in waits[1:]:
            n = nc.sync.nop()
            n.ins.sync_info = bass_rust.SyncInfo(on_wait=[w], on_update=[])
        nc.sync.drain()
        nc.all_engine_barrier()
        popped = nc._tile_sem_poison_stack.pop()
        assert popped is self._sem_poison
        nc.clear_and_free_semaphores(list(self.sems.allocated().values()))
        nc.all_engine_barrier()

    tile.TileContext._drain_and_barrier = _drain_and_barrier_split


def _split_excess_waits(nc, mybir, max_waits=1):
    """walrus can only encode a limited number of sem-waits per instruction;
    hoist excess waits onto standalone EventSemaphore instrs just before."""
    import bass_rust
    n = 0
    for bb_idx, bb in enumerate(nc.m.functions[0].blocks):
        insts = list(bb.instructions)
        out = []
        changed = False
        for inst in insts:
            si = inst.sync_info
            waits = list(si.on_wait) if si is not None else []
            if len(waits) > max_waits:
                keep = waits[-max_waits:]
                for w in waits[:-max_waits]:
                    n += 1
                    es = mybir.InstEventSemaphore(
                        name=f"wsplit_{n}_{bb_idx}",
                        engine=inst.engine,
                        ins=[], outs=[],
                        sync_info=bass_rust.SyncInfo(on_wait=[w], on_update=[]),
                    )
                    nc.register_instruction(es)
                    out.append(es)
                changed = True
                inst.sync_info = bass_rust.SyncInfo(
                    on_wait=keep, on_update=list(si.on_update))
            out.append(inst)
        if changed:
            bb.instructions = out
    return n


def _perm():
    # reference gate row order: [i f c o]; ours: [f i o c]
    return np.concatenate([
        np.arange(512, 1024),    # f
        np.arange(0, 512),       # i
        np.arange(1536, 2048),   # o
        np.arange(1024, 1536),   # c
    ])


def _host_prepare(inputs, h0, c0, weight_ih, weight_hh, bias_ih, bias_hh,
                  weight_bx, weight_bh, weight_tp_ih):
    p = _perm()
    inputs = np.ascontiguousarray(np.asarray(inputs), np.float32)
    whh = np.asarray(weight_hh)[p]
    wih = np.asarray(weight_ih)[p]
    wtp = np.asarray(weight_tp_ih)[p]
    bias = (np.asarray(bias_ih) + np.asarray(bias_hh))[p]

    w_rec_t = np.concatenate([whh, np.asarray(weight_bh)], axis=0).T   # [512, 2560]
    w_pre = np.concatenate([wih, np.asarray(weight_bx)], axis=0)       # [2560, 256]
    w_pre_t = np.zeros((384, GT), np.float32)
    w_pre_t[:I] = w_pre.T
    w_pre_t[I, :G8] = bias                                             # ones-row bias
    w_tp_t = wtp.T                                                     # [256, 2048]

    in_maps = []
    for c in range(NC):
        sl = slice(c * BL, (c + 1) * BL)
        xc = inputs[:, sl, :]
        xt = np.ascontiguousarray(xc.reshape(T * BL, I).T)             # [I, T*16]
        xt_ext = np.zeros((384, BL + T * BL), np.float32)
        xt_ext[:I, :BL] = xt[:, :BL]        # old_inputs[0] = inputs[0]
        xt_ext[:I, BL:] = xt
        xt_ext[I, :] = 1.0                  # ones feature -> bias term
        h0c0 = np.concatenate([np.asarray(h0)[sl], np.asarray(c0)[sl]],
                              axis=1).astype(np.float32)
        ht0 = np.ascontiguousarray(h0c0.T)                             # [512, 16]
        in_maps.append({
            "xt": xt_ext.astype(np.float16),
            "w_rec_t": w_rec_t.astype(np.float16),
            "w_pre_t": w_pre_t.astype(np.float16),
            "w_tp_t": w_tp_t.astype(np.float16),
            "ht0_32": ht0.astype(np.float32),
            "ht0_16": ht0.astype(np.float16),
            "ident128": np.eye(128, dtype=np.float16),
        })
    return in_maps


def _build(t_steps=T):
    import concourse.bass as bass_mod
    import concourse.tile as tile_mod
    import concourse.mybir as mybir
    from contextlib import ExitStack

    f32 = mybir.dt.float32
    f16 = mybir.dt.float16
    AF = mybir.ActivationFunctionType
    n_blocks = t_steps // TB

    nc = bass_mod.Bass("TRN2", target_bir_lowering=False, debug=False)
    xt_d = nc.dram_tensor("xt", [384, BL + T * BL], f16, kind="ExternalInput").ap()
    wrec_d = nc.dram_tensor("w_rec_t", [SD, GT], f16, kind="ExternalInput").ap()
    wpre_d = nc.dram_tensor("w_pre_t", [384, GT], f16, kind="ExternalInput").ap()
    wtp_d = nc.dram_tensor("w_tp_t", [I, G8], f16, kind="ExternalInput").ap()
    ht032_d = nc.dram_tensor("ht0_32", [SD, BL], f32, kind="ExternalInput").ap()
    ht016_d = nc.dram_tensor("ht0_16", [SD, BL], f16, kind="ExternalInput").ap()
    id128_d = nc.dram_tensor("ident128", [128, 128], f16, kind="ExternalInput").ap()
    outp_d = nc.dram_tensor("out_p", [t_steps, BL, H], f32, kind="ExternalOutput").ap()
    outh_d = nc.dram_tensor("out_h", [BL, H], f32, kind="ExternalOutput").ap()
    outc_d = nc.dram_tensor("out_c", [BL, H], f32, kind="ExternalOutput").ap()

    with tile_mod.TileContext(nc) as tc, ExitStack() as ctx:
        consts = ctx.enter_context(tc.tile_pool(name="consts", bufs=1))
        pre_pool = ctx.enter_context(tc.tile_pool(name="pre", bufs=2))
        xb_pool = ctx.enter_context(tc.tile_pool(name="xb", bufs=2))
        state_pool = ctx.enter_context(tc.tile_pool(name="state", bufs=2))
        tmp_pool = ctx.enter_context(tc.tile_pool(name="tmp", bufs=2))
        pblk_pool = ctx.enter_context(tc.tile_pool(name="pblk", bufs=2))
        psum_g = ctx.enter_context(tc.tile_pool(name="psg", bufs=2, space="PSUM"))
        psum_pre = ctx.enter_context(tc.tile_pool(name="pspre", bufs=2, space="PSUM"))
        psum_tr = ctx.enter_context(tc.tile_pool(name="pstr", bufs=2, space="PSUM"))

        wrec = consts.tile([128, KC, GT], f16)
        nc.sync.dma_start(wrec[:], wrec_d.rearrange("(c p) g -> p c g", p=128))
        wpre = consts.tile([128, 3, GT], f16)
        nc.sync.dma_start(wpre[:], wpre_d.rearrange("(c p) g -> p c g", p=128))
        wtp = consts.tile([128, 2, G8], f16)
        nc.sync.dma_start(wtp[:], wtp_d.rearrange("(c p) g -> p c g", p=128))
        id128 = consts.tile([128, 128], f16)
        nc.sync.dma_start(id128[:], id128_d[:])

        h32 = state_pool.tile([128, KC, BL], f32, tag="h32", name="h32_init")
        nc.sync.dma_start(h32[:], ht032_d.rearrange("(c p) b -> p c b", p=128))
        h16 = state_pool.tile([128, KC, BL], f16, tag="h16", name="h16_init")
        nc.sync.dma_start(h16[:], ht016_d.rearrange("(c p) b -> p c b", p=128))

        pre_bufs = {}

        def emit_pre_block(j):
            xb = xb_pool.tile([128, 3, NTB + BL], f16, tag="xb", name=f"xb{j}")
            nc.sync.dma_start(
                xb[:], xt_d[:, j * NTB: j * NTB + NTB + BL]
                .rearrange("(c p) n -> p c n", p=128))
            pre_buf = pre_pool.tile([128, MTOT, NTB], f32, tag="prebuf",
                                    name=f"pre{j}")
            for m in range(MTOT):
                ms = slice(m * 128, (m + 1) * 128)
                ps = psum_pre.tile([128, NTB], f32, tag="preps", name=f"pps{j}_{m}")
                mms = [(wpre[:, kc, ms], xb[:, kc, BL:]) for kc in range(3)]
                if m < 16:   # x_old projections only touch the gate rows
                    mms += [(wtp[:, kc, ms], xb[:, kc, :NTB]) for kc in range(2)]
                for q, (lh, rh) in enumerate(mms):
                    nc.tensor.matmul(ps[:], lh, rh,
                                     start=(q == 0), stop=(q == len(mms) - 1))
                eng = nc.scalar.copy if m % 2 else nc.vector.tensor_copy
                eng(pre_buf[:, m, :], ps[:])
            pre_bufs[j] = pre_buf

        emit_pre_block(0)
        pblk = None

        for t in range(t_steps):
            j, r = divmod(t, TB)
            pre_buf = pre_bufs[j]
            cs = slice(r * BL, (r + 1) * BL)

            gp = psum_g.tile([128, MTOT, BL], f32, tag="gp", name=f"gp{t}")
            # c and z gate tiles first, sigma gates (f, i, o) last: each chain
            # segment then overlaps the next gate group's matmuls, leaving
            # only sigma_o and one multiply after the final matmul
            for m in (12, 13, 14, 15, 16, 17, 18, 19,
                      0, 1, 2, 3, 4, 5, 6, 7, 8, 9, 10, 11):
                ms = slice(m * 128, (m + 1) * 128)
                for c in range(KC):
                    nc.tensor.matmul(gp[:, m, :], wrec[:, c, ms], h16[:, c, :],
                                     start=(c == 0), stop=(c == KC - 1))

            def gadd(name_, lo, hi):
                v = tmp_pool.tile([128, hi - lo, BL], f32, tag=name_,
                                  name=f"{name_}{t}")
                nc.vector.tensor_add(v[:], gp[:, lo:hi, :], pre_buf[:, lo:hi, cs])
                return v

            tc_ = gadd("tc", 12, 16)
            c_s = tmp_pool.tile([128, KC, BL], f32, tag="c_s", name=f"c_s{t}")
            nc.scalar.activation(c_s[:], tc_[:], AF.Tanh)
            bz = gadd("bz", 16, 20)
            acc = tmp_pool.tile([128, KC, BL], f32, tag="acc", name=f"acc{t}")
            nc.vector.tensor_add(acc[:], c_s[:], bz[:])
            tf = gadd("tf", 0, 4)
            sf = tmp_pool.tile([128, KC, BL], f32, tag="sf", name=f"sf{t}")
            nc.scalar.activation(sf[:], tf[:], AF.Sigmoid)
            m2 = tmp_pool.tile([128, KC, BL], f32, tag="m2", name=f"m2{t}")
            nc.vector.tensor_mul(m2[:], sf[:], h32[:])
            ti = gadd("ti", 4, 8)
            si = tmp_pool.tile([128, KC, BL], f32, tag="si", name=f"si{t}")
            nc.scalar.activation(si[:], ti[:], AF.Sigmoid)
            m1 = tmp_pool.tile([128, KC, BL], f32, tag="m1", name=f"m1{t}")
            nc.vector.tensor_mul(m1[:], si[:], acc[:])
            s_t = tmp_pool.tile([128, KC, BL], f32, tag="s_t", name=f"s_t{t}")
            nc.vector.tensor_add(s_t[:], m1[:], m2[:])
            ts = tmp_pool.tile([128, KC, BL], f32, tag="ts", name=f"ts{t}")
            nc.scalar.activation(ts[:], s_t[:], AF.Tanh)
            to = gadd("to", 8, 12)
            so = tmp_pool.tile([128, KC, BL], f32, tag="so", name=f"so{t}")
            nc.scalar.activation(so[:], to[:], AF.Sigmoid)
            # fp16 state first: it is the only input of step t+1's matmuls
            h16n = state_pool.tile([128, KC, BL], f16, tag="h16", name=f"h16_{t}")
            nc.vector.tensor_mul(h16n[:], so[:], ts[:])
            # fp32 copy (for next step's f*h) off the critical path
            h32n = state_pool.tile([128, KC, BL], f32, tag="h32", name=f"h32_{t}")
            nc.vector.tensor_mul(h32n[:], so[:], ts[:])
            h32, h16 = h32n, h16n

            if r == 0:
                pblk = pblk_pool.tile([128, 2, NTB], f16, tag="pblk", name=f"pb{j}")
            nc.scalar.copy(pblk[:, :, cs], h16n[:, 0:2, :])

            if r == 0 and j + 1 < n_blocks:
                emit_pre_block(j + 1)
            if r == TB - 1:
                # transpose [256(h), 256(tb)] -> [256(tb), 256(h)], then DMA
                trp = psum_tr.tile([128, 4, 128], f16, tag="trp", name=f"trp{j}")
                for cc in range(2):
                    for nn_ in range(2):
                        nc.tensor.transpose(trp[:, cc * 2 + nn_, :],
                                            pblk[:, cc, nn_ * 128:(nn_ + 1) * 128],
                                            id128[:])
                ob = pblk_pool.tile([128, 2, 256], f32, tag="ob", name=f"ob{j}")
                nc.scalar.copy(ob[:, 0, 0:128], trp[:, 0, :])
                nc.scalar.copy(ob[:, 0, 128:256], trp[:, 2, :])
                nc.scalar.copy(ob[:, 1, 0:128], trp[:, 1, :])
                nc.scalar.copy(ob[:, 1, 128:256], trp[:, 3, :])
                nc.sync.dma_start(
                    outp_d[j * TB:(j + 1) * TB]
                    .rearrange("t b h -> (t b) h")
                    .rearrange("(c p) h -> p c h", p=128), ob[:])
                pre_bufs.pop(j, None)

        fin = psum_tr.tile([16, 4, 128], f16, tag="fin", name="fin")
        for cc in range(4):
            nc.tensor.transpose(fin[:, cc, :], h16[:, cc, :], id128[:])
        hcbuf = pblk_pool.tile([16, 4, 128], f32, tag="hc", name="hcbuf")
        nc.scalar.copy(hcbuf[:], fin[:])
        nc.sync.dma_start(outh_d[:], hcbuf[:, 0:2, :].rearrange("b c h -> b (c h)"))
        nc.sync.dma_start(outc_d[:], hcbuf[:, 2:4, :].rearrange("b c h -> b (c h)"))

    _split_excess_waits(nc, mybir)
    return nc


_CACHE = {}


def kernel(**inputs):
    _install_drain_fix()
    from concourse.bass_utils import run_bass_kernel_spmd

    in_maps = _host_prepare(**inputs)
    if "nc" not in _CACHE:
        _CACHE["nc"] = _build(T)
    nc = _CACHE["nc"]
    res = run_bass_kernel_spmd(nc, in_maps, core_ids=list(range(NC)))
    outs = res.results
    got_p = np.concatenate([outs[c]["out_p"] for c in range(NC)], axis=1)
    got_h = np.concatenate([outs[c]["out_h"] for c in range(NC)], axis=0)
    got_c = np.concatenate([outs[c]["out_c"] for c in range(NC)], axis=0)
    return got_p, got_h, got_c
